# revision 49
# baseline (speedup 1.0000x reference)
"""DigitCaps dynamic-routing kernel for 8 Trainium2 NeuronCores — v4.

Problem: x(32,16384,8) f32, W(10,16384,8,16) f32 -> v(32,10,16) f32
  u_hat[b,j,p,o] = sum_d x[b,p,d] W[j,p,d,o]   (never materialized)
  3 routing iterations (softmax over j, weighted sums over p).

Measured lineage: v1 596us, v2 452us, v3 565us (gpsimd tree-adds and a
deeper DRAM scatter chain backfired). v4:
  * all matmuls bf16; W resident in SBUF (ws + wz layouts, 10.5MB)
  * s-phase: stationary = W-slice [128,(j4,o)], moving = y[128,(j4),(b)]
    emitted j-group-contiguous so LDWEIGHTS pipelines (measured
    81-151ns/mm vs 223 when psum groups alternate per-mm)
  * y kept in [p,J,D,B] order: its build mul hits the DVE broadcast
    fast path (0.56ns/elem vs 2.2 the other way)
  * consume: scalar drains 7/10 psum j-slots to bf16, DVE muls + DVE
    add-tree (gpsimd tree was 2.9ns/elem — kept to 2 muls + softmax c)
  * AllReduce carries s in [O,J,B]; squash scale computed via a
    ones-vector matmul over the o-partitions, so the v^T scatter is
    3 DMA hops (f->DRAM->bcast) + 8 replication DMAs + 1 masked mul
  * warmup collective on junk DRAM at t=0 absorbs the ncfw barrier
"""
import numpy as np
import ml_dtypes
from functools import lru_cache

import concourse.bacc as bacc
import concourse.mybir as mybir
from concourse import tile
from concourse.bass_utils import run_bass_kernel_spmd

F32 = mybir.dt.float32
BF16 = mybir.dt.bfloat16
AX = mybir.AxisListType
ALU = mybir.AluOpType
ACTF = mybir.ActivationFunctionType

B, J, P, D, O = 32, 10, 16384, 8, 16
NCORES = 8
PL = P // NCORES          # 2048
T = PL // 128             # 16 tiles of 128 p's
TG = 4                    # t-group size in z-phase
NTG = T // TG             # 4
JO = J * O                # 160
JB = J * B                # 320
JGS = [(0, 4), (4, 4), (8, 2)]   # (j0, width) j-groups for s-phase

A_SET = set(range(J))            # consume slots drained by scalar copy
AMUL_GPS = {1, 5}                # ... whose bf16 mul runs on gpsimd


def _emit(nc, n_cores):
    xb = nc.dram_tensor("xb", [128, T, D, B], BF16, kind="ExternalInput")
    x2 = nc.dram_tensor("x2", [128, T, B, D], BF16, kind="ExternalInput")
    ws = nc.dram_tensor("ws", [128, T, D, J, O], BF16, kind="ExternalInput")
    wz = nc.dram_tensor("wz", [J, 128, T, 128], BF16, kind="ExternalInput")
    vmask = nc.dram_tensor("vmask", [128, D], BF16, kind="ExternalInput")
    ones16 = nc.dram_tensor("ones16", [O, O], BF16, kind="ExternalInput")
    s3pT = nc.dram_tensor("s3pT", [O, J, B], F32, kind="ExternalOutput")

    with tile.TileContext(nc) as tc:
        with (
            tc.tile_pool(name="per", bufs=1) as per,
            tc.tile_pool(name="ypool", bufs=8) as ypool,
            tc.tile_pool(name="sm", bufs=2) as sm,
            tc.tile_pool(name="zc", bufs=2) as zc,
            tc.tile_pool(name="wzst", bufs=6) as wzst,
            tc.tile_pool(name="small", bufs=1) as small,
            tc.tile_pool(name="sps", bufs=1, space="PSUM") as sps,
            tc.tile_pool(name="zps", bufs=2, space="PSUM") as zps,
            tc.tile_pool(name="dram", bufs=2, space="DRAM") as dramp,
        ):


            # load order: what it0's s-phase needs first (x, ws), the rest
            # (x2, wz, consts) only matter ~100us in.
            x_sb = per.tile([128, T, D, B], BF16)
            nc.sync.dma_start(x_sb[:], xb[:, :, :, :])
            ws_t = []
            for t in range(T):
                w = per.tile([128, D, J, O], BF16, name=f"ws{t}")
                nc.sync.dma_start(w[:], ws[:, t, :, :, :])
                ws_t.append(w)
            x2_sb = per.tile([128, T, B, D], BF16)
            nc.sync.dma_start(x2_sb[:], x2[:, :, :, :])
            # z-phase rhs: vblk[(d,o), j, (b,d')] = v[b,j,o] iff d==d'
            vblk = per.tile([128, J, B * D], BF16)
            vrep = per.tile([128, JB], BF16)
            vmsk = per.tile([128, D], BF16)
            nc.sync.dma_start(vmsk[:], vmask[:, :])
            one_sb = per.tile([O, O], BF16)
            nc.sync.dma_start(one_sb[:], ones16[:, :])
            bb = per.tile([128, T, J, B], F32)

            y_t = [None] * T

            def allreduce(src_dram):
                out = dramp.tile([O, J, B], F32)
                nc.gpsimd.collective_compute(
                    "AllReduce", ALU.add,
                    replica_groups=[list(range(n_cores))],
                    ins=[src_dram[:].opt()], outs=[out[:].opt()],
                )
                return out

            def squash_scatter(cc_out):
                """cc_out (DRAM [O,J,B] f32 summed s) -> vblk for z-phase.

                sq[j,b] = sum_o s^2 via a ones-vector matmul over the 16
                o-partitions; the squash scale f is computed on one
                partition, broadcast back via DRAM, applied to s^T."""
                s_fT = small.tile([O, J, B], F32)
                nc.sync.dma_start(s_fT[:], cc_out[:, :, :])
                ssq = small.tile([O, JB], BF16)
                sfv = s_fT.rearrange("o j b -> o (j b)")
                nc.vector.tensor_mul(ssq[:], sfv, sfv)
                # all-ones [O,O] lhsT: every o-partition gets sum_o s^2,
                # so the whole f chain stays on-chip with no broadcast hop
                sq_ps = sps.tile([128, 512], F32, tag="s0ps", name="sq_ps")
                nc.tensor.matmul(sq_ps[0:O, 0:JB], one_sb[:], ssq[:],
                                 start=True, stop=True)
                sqv = small.tile([O, JB], F32)
                nc.vector.tensor_copy(sqv[:], sq_ps[0:O, 0:JB])
                r_ = small.tile([O, JB], F32)
                nc.scalar.activation(r_[:], sqv[:], ACTF.Sqrt)
                den = small.tile([O, JB], F32)
                nc.vector.scalar_tensor_tensor(
                    den[:], sqv[:], 1.0, r_[:], ALU.add, ALU.mult)
                rc = small.tile([O, JB], F32)
                nc.vector.reciprocal(rc[:], den[:])
                f_ = small.tile([O, JB], F32)
                nc.vector.tensor_mul(f_[:], sqv[:], rc[:])
                vT = small.tile([O, JB], BF16)
                nc.vector.tensor_mul(vT[:], sfv, f_[:])
                for d in range(D):
                    nc.sync.dma_start(vrep[d * O:(d + 1) * O, :], vT[:])
                nc.vector.tensor_mul(
                    vblk.rearrange("p j (b d) -> p j b d", d=D),
                    vrep.rearrange("p (j b) -> p j b", j=J)[:, :, :, None]
                    .broadcast_to([128, J, B, D]),
                    vmsk[:, None, None, :].broadcast_to([128, J, B, D]))

            # ---------------- it0 s-phase: c == 0.1 ----------------
            s0_ps = sps.tile([128, 512], F32, tag="s0ps")
            for t in range(T):
                for d in range(D):
                    nc.tensor.matmul(
                        s0_ps[0:B, 0:JO],
                        x_sb[:, t, d, :],
                        ws_t[t][:, d, :, :],
                        start=(t == 0 and d == 0),
                        stop=(t == T - 1 and d == D - 1),
                    )
            s_sb = small.tile([B, JO], F32)
            nc.scalar.activation(s_sb[:], s0_ps[0:B, 0:JO], ACTF.Copy,
                                 scale=0.1)
            cc0 = dramp.tile([O, J, B], F32)
            for j in range(J):
                nc.sync.dma_start(
                    cc0[:, j, :].rearrange("o b -> b o"),
                    s_sb[:, j * O:(j + 1) * O])
            squash_scatter(allreduce(cc0))

            def emit_softmax_y(tg):
                """c = softmax_j(bb) for t-group tg, then y(t) = c*x."""
                t0 = tg * TG
                e_tg = sm.tile([128, TG, J, B], BF16)
                nc.scalar.activation(e_tg[:], bb[:, t0:t0 + TG, :, :],
                                     ACTF.Exp)
                # sum over j as a contiguous add tree (strided
                # tensor_reduce measured 2.3us vs ~1.1us for this)
                es1 = sm.tile([128, TG, 5, B], BF16)
                nc.vector.tensor_add(es1[:], e_tg[:, :, 0:5, :],
                                     e_tg[:, :, 5:10, :])
                es2 = sm.tile([128, TG, 2, B], BF16)
                nc.vector.tensor_add(es2[:], es1[:, :, 0:2, :],
                                     es1[:, :, 2:4, :])
                es3 = sm.tile([128, TG, B], BF16)
                nc.vector.tensor_add(es3[:], es2[:, :, 0, :],
                                     es2[:, :, 1, :])
                se = sm.tile([128, TG, B], F32)
                nc.vector.tensor_add(se[:], es3[:], es1[:, :, 4, :])
                rec = sm.tile([128, TG, B], F32)
                nc.vector.reciprocal(rec[:], se[:])
                c_tg = sm.tile([128, TG, J, B], BF16)
                nc.gpsimd.tensor_mul(
                    c_tg[:], e_tg[:],
                    rec[:, :, None, :].broadcast_to([128, TG, J, B]))
                for t4 in range(TG):
                    t = t0 + t4
                    y = ypool.tile([128, J, D, B], BF16)
                    eng = nc.vector if t % 2 == 0 else nc.gpsimd
                    eng.tensor_mul(
                        y[:],
                        c_tg[:, t4, :, None, :].broadcast_to([128, J, D, B]),
                        x_sb[:, t, None, :, :].broadcast_to([128, J, D, B]))
                    y_t[t] = y

            def emit_z_tg(it, tg):
                """z matmuls + x-weighted d-sum for t-group tg -> bb."""
                uv_all = zc.tile([128, TG, J, B], F32)
                xv = (x2_sb[:, tg * TG:(tg + 1) * TG, :, :]
                      .rearrange("p t b d -> p (t b d)"))
                for j in range(J):
                    wzt = wzst.tile([128, TG, 128], BF16)
                    nc.sync.dma_start(wzt[:], wz[j, :, tg * TG:(tg + 1) * TG, :])
                    z_ps = zps.tile([128, TG, 256], F32)
                    for t4 in range(TG):
                        nc.tensor.matmul(
                            z_ps[:, t4, :],
                            wzt[:, t4, :],
                            vblk[:, j, :],
                            start=(t4 % 2 == 0), stop=(t4 % 2 == 1))
                    zv = z_ps.rearrange("p t bd -> p (t bd)")
                    tmp2 = zc.tile([128, TG * B * D], BF16)
                    if j in A_SET:
                        ztmp = zc.tile([128, TG * B * D], BF16)
                        nc.scalar.copy(ztmp[:], zv)
                        eng = nc.gpsimd if j in AMUL_GPS else nc.vector
                        eng.tensor_mul(tmp2[:], ztmp[:], xv)
                    else:
                        nc.vector.tensor_mul(tmp2[:], zv, xv)
                    t2v = tmp2.rearrange("p (t b d) -> p t b d", t=TG, b=B)
                    u1 = zc.tile([128, TG, B, 4], BF16)
                    nc.vector.tensor_add(u1[:], t2v[:, :, :, 0:4],
                                         t2v[:, :, :, 4:8])
                    nc.vector.tensor_reduce(
                        uv_all[:, :, j, :, None], u1[:],
                        AX.X, ALU.add)
                bb_sl = bb[:, tg * TG:(tg + 1) * TG, :, :]
                if it == 0:
                    nc.vector.tensor_copy(bb_sl, uv_all[:])
                else:
                    nc.vector.tensor_add(bb_sl, bb_sl, uv_all[:])
                emit_softmax_y(tg)

            def emit_s_chunk(ps_jg, chunk):
                """s matmuls for 4 t's, j-group-contiguous (LDW pipelines).
                stationary = ws [128,(jw,o)], moving = y [128,(jw),(b)],
                out[(j,o),(j,b)] accumulated over all (t,d)."""
                for gi, (j0, jw) in enumerate(JGS):
                    for t in range(chunk * TG, (chunk + 1) * TG):
                        for d in range(D):
                            nc.tensor.matmul(
                                ps_jg[gi][0:jw * O, 0:jw * B],
                                ws_t[t][:, d, j0:j0 + jw, :],
                                y_t[t][:, j0:j0 + jw, d, :],
                                start=(t == 0 and d == 0),
                                stop=(t == T - 1 and d == D - 1),
                            )

            def drain_s(ps_jg, dst):
                """psum diag blocks [(j,o),(j,b)] -> dst[o, j, b] DRAM."""
                for gi, (j0, jw) in enumerate(JGS):
                    zsb = small.tile([64, 128], F32, name=f"zsb{gi}")
                    nc.vector.tensor_copy(
                        zsb[0:jw * O, 0:jw * B],
                        ps_jg[gi][0:jw * O, 0:jw * B])
                    for jl in range(jw):
                        nc.sync.dma_start(
                            dst[:, j0 + jl, :],
                            zsb[jl * O:(jl + 1) * O, jl * B:(jl + 1) * B])

            # ---------------- routing iterations ----------------
            for it in range(2):
                last = (it == 1)
                ps_jg = [sps.tile([128, 512], F32, name=f"spsj{gi}")
                         for gi in range(len(JGS))]
                emit_z_tg(it, 0)
                emit_z_tg(it, 1)
                emit_z_tg(it, 2)
                emit_s_chunk(ps_jg, 0)
                emit_z_tg(it, 3)
                emit_s_chunk(ps_jg, 1)
                emit_s_chunk(ps_jg, 2)
                emit_s_chunk(ps_jg, 3)
                if last:
                    drain_s(ps_jg, s3pT)
                else:
                    cc_in = dramp.tile([O, J, B], F32)
                    drain_s(ps_jg, cc_in)
                    squash_scatter(allreduce(cc_in))
    return nc


@lru_cache(maxsize=2)
def _build(n_cores):
    nc = bacc.Bacc("TRN2", target_bir_lowering=False, debug=False,
                   num_devices=n_cores)
    _emit(nc, n_cores)
    nc.compile()
    return nc


def _prep_inputs(x, W):
    """Host-side shard + relayout. Returns list of per-core input dicts."""
    x = np.asarray(x, dtype=np.float32)
    W = np.asarray(W, dtype=np.float32)
    vm = np.zeros((128, D), np.float32)
    for d in range(D):
        vm[d * O:(d + 1) * O, d] = 1.0
    vm = vm.astype(ml_dtypes.bfloat16)
    one = np.ones((O, O), np.float32).astype(ml_dtypes.bfloat16)
    in_maps = []
    for c in range(NCORES):
        xc = x[:, c * PL:(c + 1) * PL, :]              # (B, PL, D)
        Wc = W[:, c * PL:(c + 1) * PL, :, :]           # (J, PL, D, O)
        xr = np.ascontiguousarray(
            xc.reshape(B, T, 128, D).transpose(2, 1, 3, 0))        # [128,T,D,B]
        x2r = np.ascontiguousarray(
            xc.reshape(B, T, 128, D).transpose(2, 1, 0, 3))        # [128,T,B,D]
        wsr = np.ascontiguousarray(
            Wc.reshape(J, T, 128, D, O).transpose(2, 1, 3, 0, 4))  # [128,T,D,J,O]
        wzr = np.ascontiguousarray(
            Wc.reshape(J, T, 128, D, O).transpose(0, 3, 4, 1, 2)   # j,d,o,t,p
            .reshape(J, 128, T, 128))                              # [J,(d,o),T,p]
        in_maps.append({
            "xb": xr.astype(ml_dtypes.bfloat16),
            "x2": x2r.astype(ml_dtypes.bfloat16),
            "ws": wsr.astype(ml_dtypes.bfloat16),
            "wz": wzr.astype(ml_dtypes.bfloat16),
            "vmask": vm,
            "ones16": one,
        })
    return in_maps


def _squash_np(s):
    sq = np.sum(s * s, axis=-1, keepdims=True)
    return s * (sq / ((1.0 + sq) * np.sqrt(sq)))


def kernel(x, W):
    nc = _build(NCORES)
    in_maps = _prep_inputs(x, W)
    res = run_bass_kernel_spmd(nc, in_maps, list(range(NCORES)))
    s3 = np.zeros((B, J, O), np.float64)
    for r in res.results:
        s3 += r["s3pT"].astype(np.float64).transpose(2, 1, 0)
    v = _squash_np(s3)
    return v.astype(np.float32)
